# revision 15
# baseline (speedup 1.0000x reference)
"""Trainium2 Bass kernel for nn_PermutedSparseWeight.

Math: out = P0-mix( P1-mix( X*mask ) ) where both mixes are weighted sums
over 8 block-local (64-wide) permutations.  Because every permutation maps
indices within their own 64-block, the whole computation factors into
per-block matrix sandwiches:

    out[block a, block b] = B_a @ (X*mask)[a, b] @ A_b

with B_a[j, m] = sum_p c0[p, j]·[perm0[p, j] == m]   (row mix)
and  A_b[c, k] = sum_p c1[p, k]·[perm1[p, k] == c]   (col mix).

The tiny A/B matrices (1 MB each) are assembled on the host from the
c/perm metadata; all heavy data (X 64 MB, mask 16 MB, out 64 MB) is
processed on device.  d_out is sharded 8 ways (512 rows / core, a
multiple of the 64 block size, keeping row mixes core-local); A is
replicated.

On device, per 128-row chunk (2 blocks): a fp32 matmul with the X-chunk
as the stationary operand produces the row-mixed chunk directly in
transposed layout (out1T[c, j] = sum_m Wm[m, c]·BT[m, j]), which is
exactly the lhsT layout the column-mix matmul needs — no transposes.
"""

import numpy as np

D = 4096
NP = 8
BLOCK = 64
NCORES = 8
P = 128

_CACHE = {}
PROFILE = False  # test-harness switch: capture NTFF profile on the next run
LAST = {}  # test-harness: BassKernelResults of the most recent run
# matmul dtypes, output dtype and mask handling; see build_bass
CONFIG = {"mm1": "fp16", "mm2": "fp16", "mask_u8": True, "out": "fp16"}


_MAXW = 1  # walrus ISA: instructions carry at most one sync wait command


def _patch_tile_drain():
    """The walrus codegen in this environment rejects instructions carrying
    more than _MAXW semaphore waits ("Too many sync wait commands").  Two
    patches, both semantically neutral:
      1. every instruction Tile commits with more waits gets same-engine
         no-op predecessors carrying the overflow waits (engine queues are
         in-order, so the waits still all complete before the instruction);
      2. the TileContext exit drain is split into a chain of drains."""
    import concourse.tile as tile
    import bass_rust
    from concourse.vector_clock import ScopedClock

    if getattr(tile.TileContext, "_drain_patched", False):
        return

    def _split_waits(self, inst):
        si = inst.sync_info
        waits = list(si.on_wait or []) if si else []
        if len(waits) <= _MAXW:
            return
        keep = waits[-_MAXW:]
        extra = waits[: -_MAXW]
        for i in range(0, len(extra), _MAXW):
            nop = bass_rust.InstNoOp(name=self.nc.get_next_instruction_name())
            nop.engine = inst.engine
            nop.sync_info = bass_rust.SyncInfo(
                on_wait=extra[i : i + _MAXW], on_update=[]
            )
            self.nc.register_instruction(nop, overwrite=True)
            self.nc.cur_bb.bb.add_instruction(nop)
        inst.sync_info = bass_rust.SyncInfo(
            on_wait=keep, on_update=list(si.on_update or [])
        )

    orig_add = tile.TileContext._add_instruction

    def _add_instruction(self, inst):
        if inst.engine != tile.mybir.EngineType.Unassigned:
            _split_waits(self, inst)
        orig_add(self, inst)

    def _drain_and_barrier(self, tick_clock, wait_clock):
        drain_inst = self.nc.sync.drain()
        wait_clock.add_sem_waits(
            drain_inst.ins, ScopedClock({None: tick_clock.global_clock})
        )
        si = drain_inst.ins.sync_info
        waits = list(si.on_wait or []) if si else []
        if len(waits) > _MAXW:
            drain_inst.ins.sync_info = bass_rust.SyncInfo(
                on_wait=waits[:_MAXW], on_update=list(si.on_update or [])
            )
            for i in range(_MAXW, len(waits), _MAXW):
                d2 = self.nc.sync.drain()
                si2 = d2.ins.sync_info
                upd = list(si2.on_update or []) if si2 else []
                d2.ins.sync_info = bass_rust.SyncInfo(
                    on_wait=waits[i : i + _MAXW], on_update=upd
                )
        self.nc.all_engine_barrier()
        assert self.sems is not None
        popped = self.nc._tile_sem_poison_stack.pop()
        assert popped is self._sem_poison
        self.nc.clear_and_free_semaphores(list(self.sems.allocated().values()))
        self.nc.all_engine_barrier()

    tile.TileContext._add_instruction = _add_instruction
    tile.TileContext._drain_and_barrier = _drain_and_barrier
    tile.TileContext._drain_patched = True


def build_bass(rows, d, mm1="fp16", mm2="fp16", mask_u8=True, out="fp16"):
    """One-core SPMD program: rows x d shard of X/mask -> rows x d of out.

    mm1/mm2: dtype of the row-mix / col-mix matmuls ("f32"/"bf16"/"fp16").
    out: dtype of the output DRAM tensor (16-bit halves the store traffic;
    the host upcasts back to f32).
    mask_u8: keep the mask as uint8 in SBUF and multiply directly.

    Structure: ALL input DMAs are issued up front (X + bt on the SP HWDGE
    ring, mask + amat on the ACT ring) so the two load rings stream
    back-to-back at full HBM bandwidth for the whole run; store triggers
    all live on the SP engine, which is idle after the prefetch burst, so
    a store waiting on compute never delays a load.  PSUM evictions are
    split across vector/scalar/gpsimd so no single engine becomes the
    drain bottleneck."""
    import concourse.bass as bass
    import concourse.tile as tile
    from concourse import mybir

    _patch_tile_drain()

    f32 = mybir.dt.float32
    bf16 = mybir.dt.bfloat16
    fp16 = mybir.dt.float16
    u8 = mybir.dt.uint8
    f32r = mybir.dt.float32r
    dmap = {"f32": f32, "bf16": bf16, "fp16": fp16}
    mm1_dt = dmap[mm1]
    mm2_dt = dmap[mm2]
    out_dt = dmap[out]
    m_dt = u8 if mask_u8 else bf16

    rc_n = rows // P      # row chunks per core (4)
    cch = d // P          # column chunks (32)
    grp = 4               # col chunks per PSUM bank group / 512-col strip
    gn = cch // grp       # groups per row chunk (8)
    gw = grp * P          # 512
    qw = 2 * gw           # x/out DMA piece width (1024)

    nc = bass.Bass("TRN2", target_bir_lowering=False, debug=False)
    x_d = nc.dram_tensor("x", [rows, d], f32, kind="ExternalInput").ap()
    m_d = nc.dram_tensor("m", [rows, d], u8, kind="ExternalInput").ap()
    bt_d = nc.dram_tensor("bt", [P, rc_n * P], mm1_dt, kind="ExternalInput").ap()
    a_d = nc.dram_tensor("amat", [P, d], mm2_dt, kind="ExternalInput").ap()
    o_d = nc.dram_tensor("out", [rows, d], out_dt, kind="ExternalOutput").ap()

    with tile.TileContext(nc) as tc:
        with (
            tc.tile_pool(name="const", bufs=1) as constp,
            tc.tile_pool(name="xin", bufs=16) as xp,
            tc.tile_pool(name="min", bufs=4) as mp,
            tc.tile_pool(name="wq", bufs=12) as wp,
            tc.tile_pool(name="o1", bufs=3) as o1p,
            tc.tile_pool(name="osb", bufs=2) as outp,
            tc.tile_pool(name="ps1", bufs=2, space="PSUM") as ps1p,
            tc.tile_pool(name="ps2", bufs=2, space="PSUM") as ps2p,
        ):
            # ---- prefetch: every load issued before any compute ----
            # SP ring: bt, then all X pieces in consumption order.
            # ACT ring: mask0, amat quarters interleaved, remaining masks.
            bt_t = constp.tile([P, rc_n * P], mm1_dt)
            nc.sync.dma_start(bt_t[:], bt_d[:])

            m_ts = [None] * rc_n
            amat_q = [None] * 4

            def issue_mask(rc):
                m_t = mp.tile([P, d], m_dt, name=f"m{rc}", tag="m_t")
                nc.scalar.dma_start(m_t[:], m_d[rc * P : (rc + 1) * P, :])
                m_ts[rc] = m_t

            def issue_amat(q):
                a_t = constp.tile([P, d // 4], mm2_dt, name=f"amat{q}", tag=f"amat{q}")
                nc.scalar.dma_start(a_t[:], a_d[:, q * (d // 4) : (q + 1) * (d // 4)])
                amat_q[q] = a_t

            issue_mask(0)
            issue_amat(0)
            issue_amat(1)
            issue_mask(1)
            issue_amat(2)
            issue_amat(3)
            issue_mask(2)
            issue_mask(3)

            xs = [[None] * (d // qw) for _ in range(rc_n)]
            for rc in range(rc_n):
                for j in range(d // qw):
                    x_t = xp.tile([P, qw], f32, name=f"x{rc}_{j}", tag="x_t")
                    nc.sync.dma_start(
                        x_t[:], x_d[rc * P : (rc + 1) * P, j * qw : (j + 1) * qw]
                    )
                    xs[rc][j] = x_t

            # ---- compute ----
            # Per chunk: 4 "pairs" of 1024 cols.  Each pair: two 512-wide
            # mask-multiplies (vector/gpsimd), 8 mm1 matmuls into a 2-bank
            # PSUM tile, ONE 1024-wide o1 eviction, 8 mm2 matmuls, ONE
            # 1024-wide eviction straight into the store buffer.  Stores go
            # out 2048 wide through gpsimd's SWDGE queue (3rd DMA queue),
            # so the two HWDGE rings carry loads only.
            sw = 2 * qw  # store piece width (2048)
            for rc in range(rc_n):
                rs = slice(rc * P, (rc + 1) * P)
                m_t = m_ts[rc]
                oh = [
                    outp.tile([P, sw], out_dt, name=f"oq{q}", tag=f"oq{q}")
                    for q in range(2)
                ]
                for p in range(4):
                    ps1 = ps1p.tile([P, qw], f32)
                    w_ts = []
                    for h in range(2):
                        g = 2 * p + h
                        w_t = wp.tile([P, gw], mm1_dt)
                        meng = nc.vector if h == 0 else nc.gpsimd
                        meng.tensor_mul(
                            w_t[:],
                            xs[rc][p][:, h * gw : (h + 1) * gw],
                            m_t[:, g * gw : (g + 1) * gw],
                        )
                        w_ts.append(w_t)
                    for t in range(2 * grp):
                        nc.tensor.matmul(
                            ps1[:, t * P : (t + 1) * P],
                            lhsT=w_ts[t // grp][:, (t % grp) * P : (t % grp + 1) * P],
                            rhs=bt_t[:, rc * P : (rc + 1) * P],
                            start=True,
                            stop=True,
                        )
                    o1 = o1p.tile([P, qw], mm2_dt)
                    if p % 2 == 0:
                        nc.scalar.copy(o1[:], ps1[:])
                    else:
                        nc.vector.tensor_copy(o1[:], ps1[:])
                    ps2 = ps2p.tile([P, qw], f32)
                    for t in range(2 * grp):
                        c = p * 2 * grp + t
                        aq = amat_q[c // (cch // 4)]
                        ao = (c % (cch // 4)) * P
                        nc.tensor.matmul(
                            ps2[:, t * P : (t + 1) * P],
                            lhsT=o1[:, t * P : (t + 1) * P],
                            rhs=aq[:, ao : ao + P],
                            start=True,
                            stop=True,
                        )
                    j = p // 2
                    off = (p % 2) * qw
                    if p == 1:
                        nc.vector.tensor_copy(oh[j][:, off : off + qw], ps2[:])
                    else:
                        nc.scalar.copy(oh[j][:, off : off + qw], ps2[:])
                    # 2048-wide stores through gpsimd SWDGE (3rd queue)
                    if p % 2 == 1:
                        nc.gpsimd.dma_start(o_d[rs, j * sw : (j + 1) * sw], oh[j][:])
    return nc


def host_prep(c_0, c_1, permutations_0, permutations_1, d):
    """Build the block-diagonal mix matrices.

    Returns bt_all [d//128, 128, 128] (chunk, m_local, j_local) and
    amat [128, d] (c_local, chunk*128 + k_local)."""
    k = np.arange(d)
    p0 = np.asarray(permutations_0)
    p1 = np.asarray(permutations_1)
    c0 = np.asarray(c_0, dtype=np.float32)
    c1 = np.asarray(c_1, dtype=np.float32)
    cch = d // P

    bt = np.zeros((d, BLOCK), np.float32)  # [j, m_local]
    for p in range(p0.shape[0]):
        np.add.at(bt, (k, p0[p] % BLOCK), c0[p])
    b4 = bt.reshape(cch, 2, BLOCK, BLOCK)  # [chunk, half, j_loc, m_loc]
    bt_all = np.zeros((cch, P, P), np.float32)
    bt_all[:, :BLOCK, :BLOCK] = b4[:, 0].transpose(0, 2, 1)
    bt_all[:, BLOCK:, BLOCK:] = b4[:, 1].transpose(0, 2, 1)

    a1 = np.zeros((d, BLOCK), np.float32)  # [k, c_local]
    for p in range(p1.shape[0]):
        np.add.at(a1, (k, p1[p] % BLOCK), c1[p])
    a4 = a1.reshape(cch, 2, BLOCK, BLOCK)  # [chunk, half, k_loc, c_loc]
    a_all = np.zeros((cch, P, P), np.float32)
    a_all[:, :BLOCK, :BLOCK] = a4[:, 0].transpose(0, 2, 1)
    a_all[:, BLOCK:, BLOCK:] = a4[:, 1].transpose(0, 2, 1)
    amat = np.ascontiguousarray(a_all.transpose(1, 0, 2).reshape(P, d))
    return bt_all, amat


def _numpy_fallback(X, c_0, c_1, mask, p0, p1):
    W = np.asarray(X, np.float32) * np.asarray(mask)
    W = np.einsum("ipk,pk->ik", W[:, p1], np.asarray(c_1, np.float32))
    W = np.einsum("pjk,pj->jk", W[p0, :], np.asarray(c_0, np.float32))
    return W.astype(np.float32)


def kernel(X, c_0, c_1, mask, permutations_0, permutations_1):
    X = np.asarray(X)
    mask = np.asarray(mask)
    p0 = np.asarray(permutations_0)
    p1 = np.asarray(permutations_1)

    d = X.shape[1]
    k = np.arange(d)
    block_local = (
        X.shape == (D, D)
        and p0.shape == (NP, D)
        and p1.shape == (NP, D)
        and (p0 // BLOCK == k // BLOCK).all()
        and (p1 // BLOCK == k // BLOCK).all()
    )
    if not block_local:
        return _numpy_fallback(X, c_0, c_1, mask, p0, p1)

    from concourse.bass_utils import run_bass_kernel_spmd

    rows = D // NCORES
    cfg = dict(CONFIG)
    key = ("nc", cfg["mm1"], cfg["mm2"], cfg["mask_u8"], cfg["out"])
    if key not in _CACHE:
        _CACHE[key] = build_bass(rows, D, **cfg)
    nc = _CACHE[key]

    def _mmdt(which):
        if cfg[which] == "bf16":
            import ml_dtypes

            return ml_dtypes.bfloat16
        if cfg[which] == "fp16":
            return np.float16
        return np.float32

    bt_all, amat = host_prep(c_0, c_1, p0, p1, D)
    amat = np.ascontiguousarray(amat.astype(_mmdt("mm2")))
    rc_n = rows // P
    xf = np.ascontiguousarray(X, dtype=np.float32)
    mu = np.ascontiguousarray(mask.astype(np.uint8))

    in_maps = []
    for i in range(NCORES):
        rs = slice(i * rows, (i + 1) * rows)
        bt_core = np.ascontiguousarray(
            bt_all[i * rc_n : (i + 1) * rc_n]
            .transpose(1, 0, 2)
            .reshape(P, rc_n * P)
            .astype(_mmdt("mm1"))
        )
        in_maps.append(
            {
                "x": xf[rs],
                "m": mu[rs],
                "bt": bt_core,
                "amat": amat,
            }
        )

    res = run_bass_kernel_spmd(nc, in_maps, list(range(NCORES)), trace=PROFILE)
    LAST["res"] = res
    out = np.concatenate([res.results[i]["out"] for i in range(NCORES)], axis=0)
    return out.astype(np.float32)

